# revision 6
# baseline (speedup 1.0000x reference)
"""Trainium2 Bass kernel for nn_NodeModel (GNN message passing).

  out = relu(concat([x, scatter_mean(edge_attr, col), u[batch]]) @ W1 + b1) @ W2 + b2

v4 = v3 (segment-sum folded into PE matmul accumulation, fp8 edges,
DoubleRow, degree-sorted node groups) with two byte cuts, since the
kernel is HBM-bound:

  * u[batch] is not shipped per node. Host computes hu = u @ W1u
    ([64 graphs, H] bf16, tiny) and the device adds it per node with one
    matmul against a one-hot graph-membership rhs ([64, cols] fp8):
    128B/node bf16 -> 64B/node fp8.
  * Edge capacity in 8-slot planes (ceil(maxdeg/8)) instead of 16-slot
    DoubleRow k-tiles (2*ceil(maxdeg/16)): plane pairs run as DoubleRow
    matmuls, a trailing odd plane as a plain fp8 matmul.
  * No cross-core communication: edges live with their destination node.
"""

import numpy as np

try:
    import ml_dtypes

    _BF16 = np.dtype(ml_dtypes.bfloat16)
    _FP8 = np.dtype(ml_dtypes.float8_e4m3fn)
except Exception:  # pragma: no cover
    _BF16 = None
    _FP8 = None

F_E, F_X, F_U, H, F_OUT = 16, 64, 64, 128, 64

CFG = dict(
    n_cores=8,
    n_nodes=100000,
    n_graphs=64,
    ng=25,        # groups per core
    cols=512,     # nodes per group (matmul moving dim)
    out_batch=5,  # groups per output DMA
    in_batch=5,   # groups per x/one-hot DMA
    et_chunk=5,   # groups per edge DMA
    dev_oh=False,  # one-hot via DMA beats on-device synthesis (DVE-bound)
)

_CACHE = {}


# ---------------------------------------------------------------- host side
def _plan(col, cfg):
    """Degree-sorted node permutation and per-group-slot plane schedule."""
    NC, NG, COLS = cfg["n_cores"], cfg["ng"], cfg["cols"]
    NPAD = NC * NG * COLS
    cnt = np.bincount(col, minlength=NPAD)  # pad nodes have degree 0
    order = np.argsort(cnt, kind="stable").astype(np.int64)  # ascending degree
    deg_sorted = cnt[order]
    gmax = deg_sorted.reshape(NC * NG, COLS).max(1)
    nps = np.ceil(gmax.reshape(NG, NC).max(1) / 8.0).astype(np.int64)
    nps = np.maximum(nps, 1)  # planes of 8 edge slots per group
    gi = np.arange(NPAD, dtype=np.int32) // COLS
    core = np.empty(NPAD, np.int32)
    kslot = np.empty(NPAD, np.int32)
    colidx = np.empty(NPAD, np.int32)
    core[order] = gi % NC
    kslot[order] = gi // NC
    colidx[order] = np.arange(NPAD, dtype=np.int32) % COLS
    # node_at[c, slot]: node id occupying (core c, slot k*COLS+ci)
    node_at = np.empty(NPAD, np.int64)
    pos = (gi % NC).astype(np.int64) * (NG * COLS) \
        + (gi // NC).astype(np.int64) * COLS \
        + np.arange(NPAD, dtype=np.int64) % COLS
    node_at[pos] = order
    node_at = node_at.reshape(NC, NG * COLS)
    return cnt, core, kslot, colidx, node_at, tuple(int(v) for v in nps)


def _preprocess(inputs, cfg):
    NC, NG, COLS = cfg["n_cores"], cfg["ng"], cfg["cols"]
    N, GR = cfg["n_nodes"], cfg["n_graphs"]
    SLOTS = NG * COLS

    x = np.asarray(inputs["x"], np.float32)
    ea = np.asarray(inputs["edge_attr"], np.float32)
    u = np.asarray(inputs["u"], np.float32)
    W1 = np.asarray(inputs["W1"], np.float32)
    b1 = np.asarray(inputs["b1"], np.float32)
    W2 = np.asarray(inputs["W2"], np.float32)
    b2 = np.asarray(inputs["b2"], np.float32)
    col = np.asarray(np.asarray(inputs["edge_index"])[1], np.int64)
    batch = np.asarray(inputs["batch"], np.int64)
    assert x.shape[0] == N and u.shape[0] == GR

    cnt, core, kslot, colidx, node_at, nps = _plan(col, cfg)
    cfg["nps"] = nps
    NPS = np.array(nps, np.int32)
    off = ((np.cumsum(NPS) - NPS) * COLS).astype(np.int32)  # plane offsets
    TOT = int(NPS.sum()) * COLS

    invc = np.zeros(cnt.shape[0], np.float32)
    nz = cnt > 0
    invc[nz] = 1.0 / cnt[nz]

    order = np.argsort(col, kind="stable")
    cols_s = col[order]
    eas = np.clip(ea[order] * invc[cols_s][:, None], -240.0, 240.0)
    eq = eas.astype(_FP8).view(np.uint8)  # [E, 16]

    starts = (np.cumsum(cnt) - cnt).astype(np.int64)
    rank = (np.arange(col.shape[0], dtype=np.int64) - starts[cols_s]).astype(
        np.int32)
    c = core[cols_s]
    k = kslot[cols_s]
    ci = colidx[cols_s]
    pl = rank >> 3        # plane within group
    s = rank & 7          # slot within plane

    # edges[c][part = s*16+f][off_k + pl*COLS + ci]  (fp8)
    A = np.zeros((NC, 128, TOT), np.uint8)
    free = off[k] + pl * COLS + ci
    base = (c * 128 + (s << 4)).astype(np.int64) * TOT + free
    fidx = (np.arange(F_E, dtype=np.int64) * TOT)[None, :]
    A.reshape(-1)[base[:, None] + fidx] = eq
    A = A.view(_FP8)

    # x features transposed into permuted slots (bf16), one-hot graph id (fp8)
    slot = (kslot[:N].astype(np.int64) * COLS + colidx[:N])
    xq = np.zeros((N + 1, F_X), _BF16)
    xq[:N] = x.astype(_BF16)
    nat = np.minimum(node_at, N)  # pad nodes -> zero row N
    xT = np.ascontiguousarray(
        xq[nat].transpose(0, 2, 1))  # [NC, F_X, SLOTS]
    bq = np.full(N + 1, GR, np.int32)
    bq[:N] = batch.astype(np.int32)
    if cfg.get("dev_oh", True):
        # graph-id per slot (bf16 holds ints <= 256 exactly); pads get GR,
        # which matches no iota row -> all-zero one-hot column
        gid = bq[nat].astype(np.float32).astype(_BF16)[:, None, :]  # [NC,1,SLOTS]
        oh = None
    else:
        one = np.float32(1.0).astype(_FP8).view(np.uint8).item()
        ohw = np.zeros((NC, GR + 1, SLOTS), np.uint8)
        sl = np.arange(SLOTS)
        for cidx in range(NC):
            ohw[cidx, bq[nat[cidx]], sl] = one
        oh = np.ascontiguousarray(ohw[:, :GR]).view(_FP8)
        gid = None

    # W1 rows: x 0:64, e 64:80, u 80:144
    W1x = np.ascontiguousarray(W1[0:F_X], dtype=_BF16)          # [64, H]
    hu = np.ascontiguousarray(u @ W1[F_X + F_E:], dtype=_BF16)  # [64, H]
    W1e = W1[F_X:F_X + F_E]                                     # [16, H]
    W1e8 = np.tile(np.clip(W1e, -240, 240), (8, 1))             # [128, H]
    W1e8d = np.ascontiguousarray(
        np.concatenate([W1e8, W1e8], axis=1), dtype=_FP8)       # [128, 2H] DR
    W1e8s = np.ascontiguousarray(W1e8, dtype=_FP8)              # [128, H] plain
    W2c = np.ascontiguousarray(W2, dtype=_BF16)                 # [128, 64]

    common = dict(
        w1x=W1x, hu=hu, w1e8d=W1e8d, w1e8s=W1e8s, w2=W2c,
        b1=np.ascontiguousarray(b1.reshape(H, 1), np.float32),
    )
    if cfg.get("dev_oh", True):
        common["iota"] = np.arange(GR, dtype=np.float32).reshape(GR, 1)
        common["ones"] = np.ones((1, GR), _BF16)
    in_maps = []
    for cidx in range(NC):
        im = dict(common)
        im["edges"] = np.ascontiguousarray(A[cidx])
        im["xt"] = np.ascontiguousarray(xT[cidx])
        if cfg.get("dev_oh", True):
            im["gid"] = np.ascontiguousarray(gid[cidx])
        else:
            im["oh"] = np.ascontiguousarray(oh[cidx])
        in_maps.append(im)
    meta = dict(core=core[:N], slot=slot, b2=b2)
    return in_maps, meta


def _postprocess(results, meta, cfg):
    NC, NG, COLS = cfg["n_cores"], cfg["ng"], cfg["cols"]
    SLOTS = NG * COLS
    stack = np.stack(
        [np.asarray(results[c]["outT"]).astype(np.float32) for c in range(NC)]
    )  # [NC, NG, 64, COLS]
    stack = stack.transpose(0, 2, 1, 3).reshape(NC, F_OUT, SLOTS)
    out = stack[meta["core"], :, meta["slot"]]  # [N, 64]
    out += meta["b2"][None, :]
    return out


# ------------------------------------------------------------- device side
def _build(cfg):
    import concourse.bacc as bacc
    import concourse.mybir as mybir
    import concourse.tile as tile
    from contextlib import ExitStack

    NG, COLS, GR = cfg["ng"], cfg["cols"], cfg["n_graphs"]
    NPS = list(cfg["nps"])
    assert len(NPS) == NG
    SLOTS = NG * COLS
    TOT = int(sum(NPS)) * COLS
    off = np.concatenate([[0], np.cumsum(NPS)[:-1]]) * COLS
    f32 = mybir.dt.float32
    bf16 = mybir.dt.bfloat16
    fp8 = mybir.dt.float8e4
    AF = mybir.ActivationFunctionType

    nc = bacc.Bacc("TRN2", target_bir_lowering=False)

    edges_d = nc.dram_tensor("edges", [128, TOT], fp8, kind="ExternalInput")
    xt_d = nc.dram_tensor("xt", [F_X, SLOTS], bf16, kind="ExternalInput")
    dev_oh = cfg.get("dev_oh", True)
    if dev_oh:
        gid_d = nc.dram_tensor("gid", [1, SLOTS], bf16, kind="ExternalInput")
        iota_d = nc.dram_tensor("iota", [GR, 1], f32, kind="ExternalInput")
        ones_d = nc.dram_tensor("ones", [1, GR], bf16, kind="ExternalInput")
    else:
        oh_d = nc.dram_tensor("oh", [GR, SLOTS], fp8, kind="ExternalInput")
    w1x_d = nc.dram_tensor("w1x", [F_X, H], bf16, kind="ExternalInput")
    hu_d = nc.dram_tensor("hu", [GR, H], bf16, kind="ExternalInput")
    w1e8d_d = nc.dram_tensor("w1e8d", [128, 2 * H], fp8, kind="ExternalInput")
    w1e8s_d = nc.dram_tensor("w1e8s", [128, H], fp8, kind="ExternalInput")
    w2_d = nc.dram_tensor("w2", [H, F_OUT], bf16, kind="ExternalInput")
    b1_d = nc.dram_tensor("b1", [H, 1], f32, kind="ExternalInput")
    out_d = nc.dram_tensor("outT", [NG, F_OUT, COLS], bf16, kind="ExternalOutput")

    with tile.TileContext(nc) as tc, ExitStack() as ctx:
        consts = ctx.enter_context(tc.tile_pool(name="consts", bufs=1))
        edge_pool = ctx.enter_context(
            tc.tile_pool(name="edges", bufs=cfg.get("edge_bufs", 3)))
        xt_pool = ctx.enter_context(tc.tile_pool(name="xt", bufs=3))
        oh_pool = ctx.enter_context(
            tc.tile_pool(name="oh", bufs=(2 * cfg.get("in_batch", 5))
                         if cfg.get("dev_oh", True) else 3))
        hid_pool = ctx.enter_context(tc.tile_pool(name="hid", bufs=2))
        out_pool = ctx.enter_context(tc.tile_pool(name="outs", bufs=3))
        psh_pool = ctx.enter_context(tc.tile_pool(name="psh", bufs=2, space="PSUM"))
        pso_pool = ctx.enter_context(tc.tile_pool(name="pso", bufs=2, space="PSUM"))

        w1x_t = consts.tile([F_X, H], bf16)
        nc.sync.dma_start(w1x_t[:], w1x_d[:])
        hu_t = consts.tile([GR, H], bf16)
        nc.sync.dma_start(hu_t[:], hu_d[:])
        w1e8d_t = consts.tile([128, 2 * H], fp8)
        nc.sync.dma_start(w1e8d_t[:], w1e8d_d[:])
        w1e8s_t = consts.tile([128, H], fp8)
        nc.sync.dma_start(w1e8s_t[:], w1e8s_d[:])
        w2_t = consts.tile([H, F_OUT], bf16)
        nc.sync.dma_start(w2_t[:], w2_d[:])
        b1_t = consts.tile([H, 1], f32)
        nc.sync.dma_start(b1_t[:], b1_d[:])
        if dev_oh:
            gid_t = consts.tile([1, SLOTS], bf16)
            nc.sync.dma_start(gid_t[:], gid_d[:])
            iota_t = consts.tile([GR, 1], f32)
            nc.sync.dma_start(iota_t[:], iota_d[:])
            ones_t = consts.tile([1, GR], bf16)
            nc.sync.dma_start(ones_t[:], ones_d[:])
            psb_pool = ctx.enter_context(
                tc.tile_pool(name="psb", bufs=2, space="PSUM"))

        OB = cfg.get("out_batch", 5)
        IB = cfg.get("in_batch", 5)
        EC = cfg.get("et_chunk", 1)  # groups per edge DMA
        assert NG % OB == 0 and NG % IB == 0 and NG % EC == 0
        DR = mybir.MatmulPerfMode.DoubleRow
        w1e8_v = w1e8d_t[:].rearrange("p (two h) -> p two h", two=2)
        outs = None
        xt_t = oh_t = None
        et_ch = None
        et_ch_off = 0
        for k_r in range(NG * cfg.get("reps", 1)):
            k = k_r % NG
            NP = NPS[k]
            o = int(off[k])
            comp_only = cfg.get("compute_only", False)
            if EC == 1:
                et = edge_pool.tile([128, NP * COLS], fp8)
                if comp_only:
                    nc.sync.dma_start(et[:, 0:4], edges_d[:, o:o + 4])
                else:
                    nc.sync.dma_start(et[:], edges_d[:, o:o + NP * COLS])
                eo = 0
            else:
                if k % EC == 0:
                    csz = int(sum(NPS[k:k + EC])) * COLS
                    et_ch = edge_pool.tile([128, csz], fp8)
                    if comp_only:
                        nc.sync.dma_start(et_ch[:, 0:4], edges_d[:, o:o + 4])
                    else:
                        nc.sync.dma_start(et_ch[:], edges_d[:, o:o + csz])
                    et_ch_off = o
                et = et_ch
                eo = o - et_ch_off
            ki = k % IB
            if ki == 0:
                xt_t = xt_pool.tile([F_X, IB * COLS], bf16)
                nc.gpsimd.dma_start(
                    xt_t[:], xt_d[:, k * COLS:(k + IB) * COLS])
                if not dev_oh:
                    oh_t = oh_pool.tile([GR, IB * COLS], fp8)
                    nc.gpsimd.dma_start(
                        oh_t[:], oh_d[:, k * COLS:(k + IB) * COLS])
            if dev_oh and ki == 0:
                # synthesize the whole chunk's one-hot tiles up front so the
                # PE->DVE->PE chain pipelines across the chunk
                ohg_tiles = []
                for j in range(IB):
                    psb = psb_pool.tile([GR, COLS], f32)
                    nc.tensor.matmul(
                        psb[:], ones_t[:],
                        gid_t[:, (k + j) * COLS:(k + j + 1) * COLS],
                        start=True, stop=True)
                    ohg_j = oh_pool.tile([GR, COLS], bf16)
                    nc.vector.tensor_scalar(
                        out=ohg_j[:], in0=psb[:], scalar1=iota_t[:, 0:1],
                        scalar2=None, op0=mybir.AluOpType.is_equal)
                    ohg_tiles.append(ohg_j)

            psh = psh_pool.tile([H, COLS], f32)
            for j in range(NP // 2):
                rhs = et[:, eo + 2 * j * COLS:eo + (2 * j + 2) * COLS].rearrange(
                    "p (two c) -> p two c", two=2)
                nc.tensor.matmul(
                    psh[:], w1e8_v, rhs,
                    start=(j == 0), stop=False, perf_mode=DR,
                )
            if NP % 2:
                nc.tensor.matmul(
                    psh[:], w1e8s_t[:], et[:, eo + (NP - 1) * COLS:eo + NP * COLS],
                    start=(NP == 1), stop=False,
                )
            nc.tensor.matmul(
                psh[:], w1x_t[:], xt_t[:, ki * COLS:(ki + 1) * COLS],
                start=False, stop=False)
            nc.tensor.matmul(
                psh[:],
                hu_t[:],
                ohg_tiles[ki][:] if dev_oh else oh_t[:, ki * COLS:(ki + 1) * COLS],
                start=False, stop=True)

            hid = hid_pool.tile([H, COLS], bf16)
            nc.scalar.activation(hid[:], psh[:], AF.Relu, bias=b1_t[:], scale=1.0)

            pso = pso_pool.tile([F_OUT, COLS], f32)
            nc.tensor.matmul(pso[:], w2_t[:], hid[:], start=True, stop=True)
            kb = k % OB
            if kb == 0:
                outs = out_pool.tile([F_OUT, OB * COLS], bf16)
            nc.vector.tensor_copy(outs[:, kb * COLS:(kb + 1) * COLS], pso[:])
            if kb == OB - 1:
                g0 = k - OB + 1
                nc.gpsimd.dma_start(
                    out_d[g0:k + 1].rearrange("g f c -> f g c"),
                    outs[:].rearrange("f (g c) -> f g c", g=OB))

    nc.finalize()
    return nc


def _get_program(cfg):
    key = tuple(sorted((k, v) for k, v in cfg.items()))
    if key not in _CACHE:
        _CACHE[key] = _build(cfg)
    return _CACHE[key]


def run(inputs, cfg=None, trace=False):
    from concourse.bass_utils import run_bass_kernel_spmd

    cfg = dict(CFG if cfg is None else cfg)
    in_maps, meta = _preprocess(inputs, cfg)
    nc = _get_program(cfg)
    res = run_bass_kernel_spmd(
        nc, in_maps, list(range(cfg["n_cores"])), trace=trace)
    out = _postprocess(res.results, meta, cfg)
    return out, res


def kernel(**inputs):
    return run(inputs)[0]
